# revision 4
# baseline (speedup 1.0000x reference)
"""Trainium2 Bass kernel for nn_CalibratedNorm.

The reference module collapses algebraically to a per-(sample, channel)
affine:

    out[b,c,h,w] = x[b,c,h,w] * A[b,c] + S[b,c]

where, with gs/gsh the folded global-BN scale/shift and ms/msh the folded
mean-of-group-BNs scale/shift (all tiny [C] host math):

    alpha[b] = sigmoid( sum_c (alpha_w[c]/HW) * sum_hw x[b,c,:,:] + alpha_b )
    A[b,c]   = gs[c]  + alpha[b] * (ms[c]  - gs[c])
    S[b,c]   = gsh[c] + alpha[b] * (msh[c] - gsh[c])

Strategy: data-parallel over batch, 4 samples per core on 8 cores. The
kernel is HBM-streaming-bound, so x and out travel as bf16 (the affine
is numerically benign: worst-case ~0.4% of max |out|, well inside the
2e-2 gate): 12.8 MB of HBM traffic per core instead of 25.7 MB. Per
core the bf16 x shard ([4,256,3136] = 6.4 MB) stays resident in SBUF:
load once at half-sample (0.8MB) granularity, per-channel reduce (DVE,
fp32 accumulate) chasing each load, tiny gate math (PE matmuls for the
cross-partition dot + partition broadcast), fused scale+shift
(tensor_scalar on DVE for one channel half, ACT affine for the other),
store once. All DMAs ride the HWDGE SP ring (no SWDGE): params travel
as ONE fused [128,11] fp32 table so the preamble isn't serialized on
Q7 descriptor emission.
"""

import sys

import numpy as np

for _p in ("/opt/trn_rl_repo",):
    if _p not in sys.path:
        sys.path.insert(0, _p)

import ml_dtypes

import concourse.bacc as bacc
import concourse.bass as bass
import concourse.tile as tile
from concourse import mybir
from concourse.bass_utils import run_bass_kernel_spmd
from concourse.tile import add_dep_helper

EPS = 1e-5
B, C, H, W, G = 32, 256, 56, 56, 32
HW = H * W  # 3136
NCORES = 8
BPC = B // NCORES  # samples per core: 4
HALVES = C // 128  # channel partition-tiles per sample: 2
ROWS = BPC * C  # 1024 rows of the per-core [ROWS, HW] x shard
F32 = mybir.dt.float32
BF16 = mybir.dt.bfloat16
NPAR = 2 * HALVES + 4 * HALVES + 1  # param cols: wp | tab(4 x HALVES) | ab


def build_module() -> bass.Bass:
    # Bacc (not raw Bass): its compile() pass splits multi-sem waits into
    # EventSemaphore instructions — TRN2 allows at most 1 wait per
    # compute instruction and walrus codegen hard-errors otherwise.
    nc = bacc.Bacc("TRN2")

    x_in = nc.dram_tensor("x", [ROWS, HW], BF16, kind="ExternalInput")
    par_in = nc.dram_tensor("par", [128, NPAR], F32, kind="ExternalInput")
    y_out = nc.dram_tensor("out", [ROWS, HW], BF16, kind="ExternalOutput")

    with tile.TileContext(nc) as tc:
        with (
            tc.tile_pool(name="xp", bufs=BPC) as xp,
            tc.tile_pool(name="cs", bufs=1) as cs,
            tc.tile_pool(name="wk", bufs=2) as wk,
            tc.tile_pool(name="ps", bufs=2, space="PSUM") as ps,
        ):
            # One fused param table, loaded on the same HWDGE ring ahead of
            # the bulk x loads (program order puts it first; it's 5.6 KB).
            par = cs.tile([128, NPAR], F32)
            nc.sync.dma_start(out=par, in_=par_in[:, :])
            wp = par[:, 0:HALVES]
            tab = par[:, HALVES : HALVES + 4 * HALVES].rearrange(
                "p (f h) -> p f h", f=4
            )
            ab = par[0:1, NPAR - 1 : NPAR]
            ones_row = cs.tile([1, 128], F32)
            nc.vector.memset(ones_row, 1.0)

            # row r = b*256 + h*128 + p  ->  (b, p, h, w)
            xv = x_in[:, :].rearrange("(b h p) w -> b p h w", h=HALVES, p=128)
            yv = y_out[:, :].rearrange("(b h p) w -> b p h w", h=HALVES, p=128)

            # Fully per-sample pipeline: sample b's store chases its own
            # load; no cross-sample barrier anywhere, so the DMA ring
            # never idles between the load phase and the store phase.
            loads = []
            stores = []
            for b in range(BPC):
                xt = xp.tile([128, HALVES, HW], BF16, name=f"xt{b}", tag="xt")
                sums = wk.tile([128, HALVES], F32, name=f"sums{b}", tag="sums")
                zp = ps.tile([1, 1], F32, name=f"zp{b}", tag="zp")
                # Half-sample (0.8MB) load granularity: reduce + dot-matmul
                # for half h run while half h^1 is still streaming in, so
                # the alpha chain ends shortly after the last byte lands.
                for h in range(HALVES):
                    loads.append(nc.sync.dma_start(out=xt[:, h, :], in_=xv[b][:, h, :]))
                    # Row sums via an in-place identity tensor_scalar with
                    # accum_out: DVE tensor_reduce only has a 1x uop (3.3us
                    # per half) but tensor_scalar runs 4x for packed bf16,
                    # and the fp32 accumulator rides along for free.
                    nc.vector.tensor_scalar(
                        out=xt[:, h, :], in0=xt[:, h, :],
                        scalar1=1.0, scalar2=None,
                        op0=mybir.AluOpType.mult,
                        op1=mybir.AluOpType.add,
                        accum_out=sums[:, h : h + 1],
                    )
                    # z += sum_p wp[p,h]*sums[p,h] via PSUM accumulation
                    nc.tensor.matmul(
                        zp[:, :], lhsT=wp[:, h : h + 1], rhs=sums[:, h : h + 1],
                        start=(h == 0), stop=(h == HALVES - 1),
                    )
                # alpha = sigmoid(z + alpha_b)
                al = wk.tile([1, 1], F32, name=f"al{b}", tag="al")
                nc.scalar.activation(
                    out=al, in_=zp[:, :],
                    func=mybir.ActivationFunctionType.Sigmoid,
                    bias=ab, scale=1.0,
                )
                # broadcast alpha to all partitions, move to SBUF
                bc = ps.tile([128, 1], F32, name=f"bc{b}", tag="bc")
                nc.tensor.matmul(
                    bc[:, :], lhsT=ones_row[:, :], rhs=al[:, :],
                    start=True, stop=True,
                )
                ac = wk.tile([128, 1], F32, name=f"ac{b}", tag="ac")
                nc.vector.tensor_copy(out=ac, in_=bc[:, :])

                # A = gs + alpha*dms ; S = gsh + alpha*dmsh   [128, 2]
                A = wk.tile([128, HALVES], F32, name=f"A{b}", tag="A")
                Sh = wk.tile([128, HALVES], F32, name=f"S{b}", tag="S")
                nc.vector.tensor_scalar_mul(out=A, in0=tab[:, 1, :], scalar1=ac)
                nc.vector.tensor_add(out=A, in0=A[:, :], in1=tab[:, 0, :])
                nc.vector.tensor_scalar_mul(out=Sh, in0=tab[:, 3, :], scalar1=ac)
                nc.vector.tensor_add(out=Sh, in0=Sh[:, :], in1=tab[:, 2, :])

                # Fused affine, halves split across DVE and ACT; store each
                # half as soon as its own affine is done.
                nc.vector.tensor_scalar(
                    out=xt[:, 0, :], in0=xt[:, 0, :],
                    scalar1=A[:, 0:1], scalar2=Sh[:, 0:1],
                    op0=mybir.AluOpType.mult, op1=mybir.AluOpType.add,
                )
                stores.append(nc.sync.dma_start(out=yv[b][:, 0, :], in_=xt[:, 0, :]))
                nc.scalar.activation(
                    out=xt[:, 1, :], in_=xt[:, 1, :],
                    func=mybir.ActivationFunctionType.Identity,
                    bias=Sh[:, 1:2], scale=A[:, 1:2],
                )
                stores.append(nc.sync.dma_start(out=yv[b][:, 1, :], in_=xt[:, 1, :]))

            # Keep every load ahead of every store in the HWDGE ring:
            # ordering-only edges (no sems) from each store to the last
            # load. Without this the scheduler interleaves stores before
            # the last load, which delays its reduce/affine by ~30us.
            for st in stores:
                add_dep_helper(
                    st.ins, loads[-1].ins, sync=False,
                    reason="loads drain before stores on SP ring",
                )

    nc.compile()
    return nc


_NC_CACHE: list = []


def _get_module() -> bass.Bass:
    if not _NC_CACHE:
        _NC_CACHE.append(build_module())
    return _NC_CACHE[0]


def _prep_in_maps(inputs: dict) -> list[dict]:
    x = np.ascontiguousarray(np.asarray(inputs["x"], dtype=np.float32))
    alpha_w = np.asarray(inputs["alpha_w"], dtype=np.float32)
    alpha_b = np.asarray(inputs["alpha_b"], dtype=np.float32)
    g_w = np.asarray(inputs["g_w"], dtype=np.float32)
    g_b = np.asarray(inputs["g_b"], dtype=np.float32)
    g_rm = np.asarray(inputs["g_rm"], dtype=np.float32)
    g_rv = np.asarray(inputs["g_rv"], dtype=np.float32)
    grp_w = np.asarray(inputs["grp_w"], dtype=np.float32)
    grp_b = np.asarray(inputs["grp_b"], dtype=np.float32)
    grp_rm = np.asarray(inputs["grp_rm"], dtype=np.float32)
    grp_rv = np.asarray(inputs["grp_rv"], dtype=np.float32)

    gs = g_w / np.sqrt(g_rv + EPS)
    gsh = g_b - g_rm * gs
    sg = grp_w / np.sqrt(grp_rv + EPS)  # [G, C]
    ms = sg.mean(axis=0)
    msh = (grp_b - grp_rm * sg).mean(axis=0)
    dms = ms - gs
    dmsh = msh - gsh

    ch = (np.arange(HALVES)[None, :] * 128 + np.arange(128)[:, None])  # [128, HALVES]
    par = np.zeros((128, NPAR), dtype=np.float32)
    par[:, 0:HALVES] = alpha_w[ch] / np.float32(HW)  # wp
    par[:, HALVES + 0 * HALVES : HALVES + 1 * HALVES] = gs[ch]
    par[:, HALVES + 1 * HALVES : HALVES + 2 * HALVES] = dms[ch]
    par[:, HALVES + 2 * HALVES : HALVES + 3 * HALVES] = gsh[ch]
    par[:, HALVES + 3 * HALVES : HALVES + 4 * HALVES] = dmsh[ch]
    par[0, NPAR - 1] = alpha_b.reshape(-1)[0]

    xb = x.reshape(NCORES, ROWS, HW).astype(ml_dtypes.bfloat16)
    in_maps = []
    for k in range(NCORES):
        in_maps.append({"x": xb[k], "par": par})
    return in_maps


def _run(inputs: dict, trace: bool = False, trace_cores=None):
    nc = _get_module()
    in_maps = _prep_in_maps(inputs)
    res = run_bass_kernel_spmd(
        nc, in_maps, core_ids=list(range(NCORES)), trace=trace,
        trace_cores=trace_cores,
    )
    outs = [
        np.asarray(r["out"]).astype(np.float32).reshape(BPC, C, H, W)
        for r in res.results
    ]
    full = np.concatenate(outs, axis=0)
    return full, res


def kernel(**inputs) -> np.ndarray:
    out, _ = _run(inputs, trace=False)
    return out


# revision 5
# speedup vs baseline: 1.3040x; 1.3040x over previous
"""Trainium2 Bass kernel for nn_CalibratedNorm.

The reference module collapses algebraically to a per-(sample, channel)
affine:

    out[b,c,h,w] = x[b,c,h,w] * A[b,c] + S[b,c]

where, with gs/gsh the folded global-BN scale/shift and ms/msh the folded
mean-of-group-BNs scale/shift (all tiny [C] host math):

    alpha[b] = sigmoid( sum_c (alpha_w[c]/HW) * sum_hw x[b,c,:,:] + alpha_b )
    A[b,c]   = gs[c]  + alpha[b] * (ms[c]  - gs[c])
    S[b,c]   = gsh[c] + alpha[b] * (msh[c] - gsh[c])

Strategy: data-parallel over batch, 4 samples per core on 8 cores. The
kernel is HBM-streaming-bound, so x and out travel as bf16 (worst-case
~0.5% of max |out|, well inside the 2e-2 gate): 12.8 MB of HBM traffic
per core. Per core the bf16 x shard ([4,256,3136] = 6.4 MB) stays
resident in SBUF: load once at half-sample (0.8MB) granularity on the
HWDGE SP ring, store once behind the loads (ordering edges keep every
load ahead of every store so alphas resolve ASAP).

The gate dot z_b = sum_c wp[c] * sum_hw x[b,c,:] runs on the otherwise
idle PE: 14 accumulating chunk-matmuls (lhsT = bf16 wp column, rhs =
[128,448] x chunks) collapse both the channel (partition) axis and 7x
of the free axis into one PSUM row [1,448]; ACT finishes it with a
Copy+accum (448 elems) and the sigmoid. This sidesteps the DVE/ACT
accumulator paths, which all run at 1x (~3.1-3.5us per half) - measured;
DVE tensor_reduce likewise only has a 1x uop. DVE is left with just the
fused scale+shift tensor_scalar ops, which hit the 4x bf16 mode
(~1.1us per half), so every engine sits far below the ~31us DMA ring
floor and the ring never starves.
"""

import sys

import numpy as np

for _p in ("/opt/trn_rl_repo",):
    if _p not in sys.path:
        sys.path.insert(0, _p)

import ml_dtypes

import concourse.bacc as bacc
import concourse.bass as bass
import concourse.tile as tile
from concourse import mybir
from concourse.bass_utils import run_bass_kernel_spmd
from concourse.tile import add_dep_helper

EPS = 1e-5
B, C, H, W, G = 32, 256, 56, 56, 32
HW = H * W  # 3136
NCORES = 8
BPC = B // NCORES  # samples per core: 4
HALVES = C // 128  # channel partition-tiles per sample: 2
ROWS = BPC * C  # 1024 rows of the per-core [ROWS, HW] x shard
F32 = mybir.dt.float32
BF16 = mybir.dt.bfloat16
NPAR = 4 * HALVES + 1  # fp32 param cols: tab(4 x HALVES) | ab
CH = 448  # gate-matmul chunk: 7 chunks x 448 = 3136, fits one PSUM bank
NCH = HW // CH


def build_module() -> bass.Bass:
    # Bacc (not raw Bass): its compile() pass splits multi-sem waits into
    # EventSemaphore instructions — TRN2 allows at most 1 wait per
    # compute instruction and walrus codegen hard-errors otherwise.
    nc = bacc.Bacc("TRN2")

    x_in = nc.dram_tensor("x", [ROWS, HW], BF16, kind="ExternalInput")
    par_in = nc.dram_tensor("par", [128, NPAR], F32, kind="ExternalInput")
    wpb_in = nc.dram_tensor("wpb", [128, HALVES], BF16, kind="ExternalInput")
    y_out = nc.dram_tensor("out", [ROWS, HW], BF16, kind="ExternalOutput")

    with tile.TileContext(nc) as tc:
        with (
            tc.tile_pool(name="xp", bufs=BPC) as xp,
            tc.tile_pool(name="cs", bufs=1) as cs,
            tc.tile_pool(name="wk", bufs=2) as wk,
            tc.tile_pool(name="ps", bufs=2, space="PSUM") as ps,
        ):
            # Tiny param tables ride the ACT HWDGE ring so the SP ring's
            # first descriptors are bulk x loads.
            par = cs.tile([128, NPAR], F32)
            nc.scalar.dma_start(out=par, in_=par_in[:, :])
            wpb = cs.tile([128, HALVES], BF16)
            nc.scalar.dma_start(out=wpb, in_=wpb_in[:, :])
            tab = par[:, 0 : 4 * HALVES].rearrange("p (f h) -> p f h", f=4)
            ab = par[0:1, NPAR - 1 : NPAR]
            ones_row = cs.tile([1, 128], F32)
            nc.vector.memset(ones_row, 1.0)

            # row r = b*256 + h*128 + p  ->  (b, p, h, w)
            xv = x_in[:, :].rearrange("(b h p) w -> b p h w", h=HALVES, p=128)
            yv = y_out[:, :].rearrange("(b h p) w -> b p h w", h=HALVES, p=128)

            # Fully per-sample pipeline: sample b's store chases its own
            # load; no cross-sample barrier anywhere, so the DMA ring
            # never idles between the load phase and the store phase.
            loads = []
            stores = []
            for b in range(BPC):
                xt = xp.tile([128, HALVES, HW], BF16, name=f"xt{b}", tag="xt")
                zrow = ps.tile([1, CH], F32, name=f"zr{b}", tag="zr")
                # z_b accumulates on PE: both the channel (partition) axis
                # and 7x of the free axis collapse into one PSUM row.
                for h in range(HALVES):
                    loads.append(nc.sync.dma_start(out=xt[:, h, :], in_=xv[b][:, h, :]))
                    for c in range(NCH):
                        nc.tensor.matmul(
                            zrow[:, :],
                            lhsT=wpb[:, h : h + 1],
                            rhs=xt[:, h, c * CH : (c + 1) * CH],
                            start=(h == 0 and c == 0),
                            stop=(h == HALVES - 1 and c == NCH - 1),
                        )
                # Finish the free axis on ACT (448 elems), then the gate.
                zscr = wk.tile([1, CH], F32, name=f"zs{b}", tag="zs")
                z = wk.tile([1, 1], F32, name=f"z{b}", tag="z")
                nc.scalar.activation(
                    out=zscr, in_=zrow[:, :],
                    func=mybir.ActivationFunctionType.Copy,
                    accum_out=z,
                )
                # alpha = sigmoid(z + alpha_b)
                al = wk.tile([1, 1], F32, name=f"al{b}", tag="al")
                nc.scalar.activation(
                    out=al, in_=z,
                    func=mybir.ActivationFunctionType.Sigmoid,
                    bias=ab, scale=1.0,
                )
                # broadcast alpha to all partitions, move to SBUF
                bc = ps.tile([128, 1], F32, name=f"bc{b}", tag="bc")
                nc.tensor.matmul(
                    bc[:, :], lhsT=ones_row[:, :], rhs=al[:, :],
                    start=True, stop=True,
                )
                ac = wk.tile([128, 1], F32, name=f"ac{b}", tag="ac")
                nc.vector.tensor_copy(out=ac, in_=bc[:, :])

                # A = gs + alpha*dms ; S = gsh + alpha*dmsh   [128,1] each
                A = wk.tile([128, HALVES], F32, name=f"A{b}", tag="A")
                Sh = wk.tile([128, HALVES], F32, name=f"S{b}", tag="S")
                for h in range(HALVES):
                    nc.vector.tensor_scalar(
                        out=A[:, h : h + 1], in0=tab[:, 1, h : h + 1],
                        scalar1=ac, scalar2=tab[:, 0, h : h + 1],
                        op0=mybir.AluOpType.mult, op1=mybir.AluOpType.add,
                    )
                    nc.vector.tensor_scalar(
                        out=Sh[:, h : h + 1], in0=tab[:, 3, h : h + 1],
                        scalar1=ac, scalar2=tab[:, 2, h : h + 1],
                        op0=mybir.AluOpType.mult, op1=mybir.AluOpType.add,
                    )

                # Fused affine on DVE (4x bf16 tensor_scalar); store each
                # half as soon as its own affine is done.
                for h in range(HALVES):
                    nc.vector.tensor_scalar(
                        out=xt[:, h, :], in0=xt[:, h, :],
                        scalar1=A[:, h : h + 1], scalar2=Sh[:, h : h + 1],
                        op0=mybir.AluOpType.mult, op1=mybir.AluOpType.add,
                    )
                    stores.append(
                        nc.sync.dma_start(out=yv[b][:, h, :], in_=xt[:, h, :])
                    )

            # Keep every load ahead of every store in the HWDGE ring:
            # ordering-only edges (no sems) from each store to the last
            # load. Without this the scheduler interleaves stores before
            # the last load, which delays the last alphas by ~10us.
            for st in stores:
                add_dep_helper(
                    st.ins, loads[-1].ins, sync=False,
                    reason="loads drain before stores on SP ring",
                )

    nc.compile()
    return nc


_NC_CACHE: list = []


def _get_module() -> bass.Bass:
    if not _NC_CACHE:
        _NC_CACHE.append(build_module())
    return _NC_CACHE[0]


def _prep_in_maps(inputs: dict) -> list[dict]:
    x = np.ascontiguousarray(np.asarray(inputs["x"], dtype=np.float32))
    alpha_w = np.asarray(inputs["alpha_w"], dtype=np.float32)
    alpha_b = np.asarray(inputs["alpha_b"], dtype=np.float32)
    g_w = np.asarray(inputs["g_w"], dtype=np.float32)
    g_b = np.asarray(inputs["g_b"], dtype=np.float32)
    g_rm = np.asarray(inputs["g_rm"], dtype=np.float32)
    g_rv = np.asarray(inputs["g_rv"], dtype=np.float32)
    grp_w = np.asarray(inputs["grp_w"], dtype=np.float32)
    grp_b = np.asarray(inputs["grp_b"], dtype=np.float32)
    grp_rm = np.asarray(inputs["grp_rm"], dtype=np.float32)
    grp_rv = np.asarray(inputs["grp_rv"], dtype=np.float32)

    gs = g_w / np.sqrt(g_rv + EPS)
    gsh = g_b - g_rm * gs
    sg = grp_w / np.sqrt(grp_rv + EPS)  # [G, C]
    ms = sg.mean(axis=0)
    msh = (grp_b - grp_rm * sg).mean(axis=0)
    dms = ms - gs
    dmsh = msh - gsh

    ch = (np.arange(HALVES)[None, :] * 128 + np.arange(128)[:, None])  # [128, HALVES]
    par = np.zeros((128, NPAR), dtype=np.float32)
    par[:, 0 * HALVES : 1 * HALVES] = gs[ch]
    par[:, 1 * HALVES : 2 * HALVES] = dms[ch]
    par[:, 2 * HALVES : 3 * HALVES] = gsh[ch]
    par[:, 3 * HALVES : 4 * HALVES] = dmsh[ch]
    par[0, NPAR - 1] = alpha_b.reshape(-1)[0]
    wpb = (alpha_w[ch] / np.float32(HW)).astype(ml_dtypes.bfloat16)

    xb = x.reshape(NCORES, ROWS, HW).astype(ml_dtypes.bfloat16)
    in_maps = []
    for k in range(NCORES):
        in_maps.append({"x": xb[k], "par": par, "wpb": wpb})
    return in_maps


def _run(inputs: dict, trace: bool = False, trace_cores=None):
    nc = _get_module()
    in_maps = _prep_in_maps(inputs)
    res = run_bass_kernel_spmd(
        nc, in_maps, core_ids=list(range(NCORES)), trace=trace,
        trace_cores=trace_cores,
    )
    outs = [
        np.asarray(r["out"]).astype(np.float32).reshape(BPC, C, H, W)
        for r in res.results
    ]
    full = np.concatenate(outs, axis=0)
    return full, res


def kernel(**inputs) -> np.ndarray:
    out, _ = _run(inputs, trace=False)
    return out
